# revision 52
# baseline (speedup 1.0000x reference)
"""Trainium2 Bass kernel for decomposed-rel-pos attention (B=4, H=W=32, DIM=768, HEADS=12).

Sharding: 48 (batch, head) pairs -> 8 cores x 6 heads (core c: batch c//2,
heads (c%2)*6 .. +6). All matmul operands in bf16 (fp32 PSUM accumulation),
bf16 output partials summed on host in fp32; end-to-end rel err vs the fp32
jax reference ~4.5e-3. TimelineSim estimate 100.6us/core (baseline 133.0).

Structure per core:
 - qk projection per head (bf16, 6 k-tiles, no bias row; qkv bias applied by
   the DVE/ACT extraction copies), rel-pos table matmuls reuse the scaled-q
   rows of the S moving operand directly.
 - S matmul folds the decomposed rel-pos bias in as extra contraction rows
   (0/1 expander stationary rows, DMA-preloaded once into both rhs tiles).
 - exp on ACT ([128,1024] chunks, psum->sbuf bf16), no max-subtraction; the
   48-exp chain is the pacing backbone (~50us), so all injected work is
   scheduled around keeping it dense.
 - AV runs with the attn tile *stationary* and V (+ones column for the row
   sums) *moving*: 65 streamed columns per (kb, qb) instead of 1024 -> half
   the PE time of the classic layout. AV psum groups are 2KB-bank aligned
   (qb 0-3 bank0 at cols qb*65, qb 4-7 bank1 at 512+(qb-4)*65) because psum
   accumulation start/stop is zero-region (bank) granular.
 - per-head tail: one snapshot copy releases the AV psum early, softmax
   normalization is a per-partition scalar multiply split DVE/Pool.
 - normalized [token, ch] pair-of-heads blocks are transposed back ch-major
   via single XBAR dma-transposes (3D out AP; no PE or PSUM), issued as soon
   as each pair of heads completes so HWDGE overhead hides under the ladders
   (the last pair in three pieces interleaved with its norms).
 - input DMA: wqk arrives in three head-pair column chunks (512B descriptors)
   with the later chunks queued behind x on the sync engine, so the first qk
   projection is gated only by the x transfer itself.
 - head-projection prefetches its t0/t1 accumulations into 4 rotating psum
   slots during the last ladder; output copies split ACT||DVE.
 - next head's qk/extract/rel phases are injected into the current head's
   ladder at fixed kb slots tuned so the serial DVE chain (extract-q/k, two
   rel-table copies) completes just before the next head's first S matmul.

PSUM: S double-buffer (4 banks) + qk/rel shared tile (2) + AV accumulator
(2) = 8 banks exactly; rel-pos table matmuls write into partition rows 0:64
of the (already consumed) qk psum tile.
"""
from contextlib import ExitStack

import numpy as np
import ml_dtypes

import concourse.bass as bass
import concourse.bacc as bacc
import concourse.mybir as mybir
import concourse.tile as tile
from concourse.bass_utils import run_bass_kernel_spmd

B, H, W, DIM, HEADS = 4, 32, 32, 768, 12
HD = DIM // HEADS  # 64
N = H * W  # 1024
HPC = HEADS // 2  # heads per core = 6
NCORES = 8
F32 = mybir.dt.float32
BF16 = mybir.dt.bfloat16
EXPF = mybir.ActivationFunctionType.Exp
IDENT = mybir.ActivationFunctionType.Identity
MUL = mybir.AluOpType.mult
ADD = mybir.AluOpType.add

_cache = {}
PHASES = []  # (label, next_instruction_number) — filled during build for tracing


def _mark(nc, label):
    PHASES.append((label, int(nc.get_next_instruction_name()[2:])))


def build_program(reps=1):
    nc = bacc.Bacc("TRN2", target_bir_lowering=False, debug=False,
                   enable_asserts=False, num_devices=NCORES)
    xT = nc.dram_tensor("xT", [DIM, N], BF16, kind="ExternalInput")
    wqk = nc.dram_tensor("wqk", [DIM, HPC * 128], BF16, kind="ExternalInput")
    wv = nc.dram_tensor("wv", [DIM, HPC * 64], BF16, kind="ExternalInput")
    wp = nc.dram_tensor("wp", [HPC * HD, DIM], BF16, kind="ExternalInput")
    rhT = nc.dram_tensor("rhT", [HD, N], BF16, kind="ExternalInput")
    rwT = nc.dram_tensor("rwT", [HD, N], BF16, kind="ExternalInput")
    ecomb = nc.dram_tensor("ecomb", [64, N], BF16, kind="ExternalInput")
    qkb = nc.dram_tensor("qkb", [128, HPC], F32, kind="ExternalInput")
    out_d = nc.dram_tensor("out_part", [N, DIM], BF16, kind="ExternalOutput")

    with ExitStack() as ctx:
        tc = ctx.enter_context(tile.TileContext(nc))
        _body(nc, tc, ctx, xT, wqk, wv, wp, rhT, rwT, ecomb, qkb, out_d)
    nc.compile()
    return nc


def _body(nc, tc, ctx, xT, wqk, wv, wp, rhT, rwT, ecomb, qkb, out_d):
    persist = ctx.enter_context(tc.tile_pool(name="persist", bufs=1))
    small = ctx.enter_context(tc.tile_pool(name="small", bufs=2))
    outp = ctx.enter_context(tc.tile_pool(name="outp", bufs=4))
    ps_s = ctx.enter_context(tc.tile_pool(name="ps_s", bufs=2, space="PSUM"))
    ps_qk = ctx.enter_context(tc.tile_pool(name="ps_qk", bufs=1, space="PSUM"))
    ps_av = ctx.enter_context(tc.tile_pool(name="ps_av", bufs=1, space="PSUM"))

    # ---- static SBUF tiles ----
    xt_sb = persist.tile([128, 6, N], BF16, tag="xt", name="xt")
    wqk_sb = persist.tile([128, 6, HPC * 128], BF16, tag="wqk", name="wqk")
    wv_sb = persist.tile([128, 6, HPC * 64], BF16, tag="wv", name="wv")
    wp_sb = persist.tile([128, 3, DIM], BF16, tag="wp", name="wp")
    rhT_sb = persist.tile([HD, N], BF16, tag="rhT", name="rhT")
    rwT_sb = persist.tile([HD, N], BF16, tag="rwT", name="rwT")
    qkb_sb = persist.tile([128, HPC], F32, tag="qkb", name="qkb")
    # double-buffered (by head parity) S operands; expander rows of rhs are
    # DMA-preloaded once and never rewritten
    lhsT = [persist.tile([128, N], BF16, tag=f"lhsT{p}", name=f"lhsT{p}") for p in range(2)]
    rhs_c = [persist.tile([128, N], BF16, tag=f"rhs{p}", name=f"rhs{p}") for p in range(2)]
    v_sb = [persist.tile([128, HPC * 65], BF16, tag=f"v{m}", name=f"v{m}") for m in range(8)]
    attnT = [persist.tile([128, N], BF16, tag=f"attnT{kb}", name=f"attnT{kb}") for kb in range(8)]
    attn_out = [persist.tile([128, N], BF16, tag=f"ao{j}", name=f"ao{j}") for j in range(3)]
    proj_lhsT = [persist.tile([128, N], BF16, tag=f"pl{j}", name=f"pl{j}") for j in range(3)]

    # ---- input DMA (issue order == consumption order) ----
    xt3 = xT.rearrange("(kt p) c -> p kt c", p=128)
    wqk3 = wqk.rearrange("(kt p) c -> p kt c", p=128)
    # wqk in three column chunks (heads 2c,2c+1 = 512B descriptors): head 0-1
    # weights (chunk 0) land in ~1.1us so qk0 is gated only by x itself; the
    # later chunks queue on the sync engine BEHIND x so they cannot steal the
    # shared DMA engines from the gating transfers.
    nc.scalar.dma_start(wqk_sb[:, :, 0:256], wqk3[:, :, 0:256])
    for kt in range(6):
        nc.sync.dma_start(xt_sb[:, kt, :], xt3[:, kt, :])
    nc.scalar.dma_start(qkb_sb[:], qkb[:])
    nc.sync.dma_start(wqk_sb[:, :, 256:512], wqk3[:, :, 256:512])
    nc.sync.dma_start(wqk_sb[:, :, 512:768], wqk3[:, :, 512:768])
    nc.sync.dma_start(rhT_sb[:], rhT[:])
    nc.sync.dma_start(rwT_sb[:], rwT[:])
    nc.sync.dma_start(wv_sb[:], wv.rearrange("(kt p) c -> p kt c", p=128))
    nc.sync.dma_start(rhs_c[0][64:128, :], ecomb[:])
    nc.sync.dma_start(rhs_c[1][64:128, :], ecomb[:])
    nc.scalar.dma_start(wp_sb[:], wp.rearrange("(t p) c -> p t c", p=128))

    # ones columns of V (row-sum trick), written once
    for m in range(8):
        v3 = v_sb[m][:].rearrange("p (h c) -> p h c", c=65)
        nc.gpsimd.memset(v3[:, :, 64], 1.0)

    # ---- per-head phases ----
    def phase_qk(h, kts):
        """qk projection for head h, k-tiles kts, into the shared qk psum."""
        _mark(nc, f"qk{h}")
        if kts[0] == 0:
            pq = ps_qk.tile([128, N], F32, tag="pqk", name="pqk")
            phase_qk.cur = pq
        pq = phase_qk.cur
        for kt in kts:
            for half in range(2):
                sl = slice(half * 512, half * 512 + 512)
                nc.tensor.matmul(pq[:, sl], wqk_sb[:, kt, h * 128:(h + 1) * 128],
                                 xt_sb[:, kt, sl], start=(kt == 0), stop=(kt == 5))
        return pq

    def phase_extract(h, pq):
        _mark(nc, f"extract{h}")
        p = h % 2
        # q rows: out = in*0.125 + qkv_b_q*0.125 ; k rows: out = in + qkv_b_k
        nc.vector.tensor_scalar(lhsT[p][0:64, :], pq[0:64, :],
                                0.125, qkb_sb[0:64, h:h + 1], MUL, ADD)
        nc.vector.tensor_scalar(rhs_c[p][0:64, :], pq[64:128, :],
                                qkb_sb[64:128, h:h + 1], None, ADD)

    def phase_rel_h(h, pq):
        _mark(nc, f"relh{h}")
        p = h % 2
        prh = pq[0:32, :]
        for qh in range(32):
            sl = slice(qh * 32, qh * 32 + 32)
            nc.tensor.matmul(prh[:, sl], rhT_sb[:, sl], lhsT[p][0:64, sl],
                             start=True, stop=True)

    def phase_rel_w(h, pq):
        _mark(nc, f"relw{h}")
        p = h % 2
        prw = pq[32:64, :]
        qT3 = lhsT[p][0:64, :].rearrange("p (a b) -> p b a", b=32)  # [64, qw, qh]
        for qw in range(32):
            sl = slice(qw * 32, qw * 32 + 32)
            nc.tensor.matmul(prw[:, sl], rwT_sb[:, sl], qT3[:, qw, :],
                             start=True, stop=True)

    def phase_rel_copy_h(h, pq):
        _mark(nc, f"relch{h}")
        nc.vector.tensor_copy(lhsT[h % 2][64:96, :], pq[0:32, :])

    def phase_rel_copy_w(h, pq):
        _mark(nc, f"relcw{h}")
        prw_v = pq[32:64, :].rearrange("p (a b) -> p b a", b=32)
        nc.vector.tensor_copy(lhsT[h % 2][96:128, :], prw_v[:, :, :])

    def phase_v(m, g=0):
        _mark(nc, f"vproj{m}")
        pv = ps_s.tile([128, HPC * 64], F32, tag="ps", name="pv")
        for kt in range(6):
            nc.tensor.matmul(pv[:], xt_sb[:, kt, m * 128:(m + 1) * 128],
                             wv_sb[:, kt, :], start=(kt == 0), stop=(kt == 5))
        dst = v_sb[m][:].rearrange("p (h c) -> p h c", c=65)[:, :, 0:64]
        nc.vector.tensor_copy(dst, pv[:].rearrange("p (h c) -> p h c", c=64))

    # AV psum layout: qb 0-3 at cols qb*65 (bank 0), qb 4-7 at 512+(qb-4)*65
    # (bank 1) so accumulation groups never straddle a 2KB psum bank; the
    # start/stop flags are bank-granular (first/last matmul touching the bank).
    def avcol(qb):
        return qb * 65 if qb < 4 else 512 + (qb - 4) * 65

    # ---- attention ladder for one head ----
    def S_unit(h, kb):
        _mark(nc, f"S{h}.{kb}")
        p = h % 2
        ps = ps_s.tile([128, N], F32, tag="ps", name="s_ps")
        for half in range(2):
            sl = slice(half * 512, half * 512 + 512)
            nc.tensor.matmul(ps[:, sl], rhs_c[p][:, kb * 128:(kb + 1) * 128],
                             lhsT[p][:, sl], start=True, stop=True)
        nc.scalar.activation(attnT[kb][:], ps[:], EXPF)

    def ladder(h, inject):
        # kb 0..2 S-units were already emitted in the previous head's tail
        # (prologue) so the exp backbone has no head-boundary bubble.
        pav = ps_av.tile([128, 512 + 4 * 65], F32, tag="pav", name="pav")

        def AV_unit(kb):
            _mark(nc, f"AV{h}.{kb}")
            for qb in range(8):
                c = avcol(qb)
                nc.tensor.matmul(pav[:, c:c + 65],
                                 attnT[kb][:, qb * 128:(qb + 1) * 128],
                                 v_sb[kb][:, h * 65:(h + 1) * 65],
                                 start=(kb == 0 and qb % 4 == 0),
                                 stop=(kb == 7 and qb % 4 == 3))

        for kb in range(8):
            if kb > 0 or h == 0:
                S_unit(h, kb)
            if kb >= 3:
                AV_unit(kb - 3)
            inject(kb)
        return pav

    def tail(h, pav, last=False):
        _mark(nc, f"tail{h}")
        if h + 1 < HPC:
            S_unit(h + 1, 0)
        for kb in (5, 6, 7):
            for qb in range(8):
                c = avcol(qb)
                nc.tensor.matmul(pav[:, c:c + 65],
                                 attnT[kb][:, qb * 128:(qb + 1) * 128],
                                 v_sb[kb][:, h * 65:(h + 1) * 65],
                                 start=False, stop=(kb == 7 and qb % 4 == 3))
        # snapshot the AV accumulator to SBUF so the psum tile is released
        # after ~1us and the Pool engine can join the normalization. The last
        # head normalizes straight from psum on the DVE (latency matters more
        # than psum hand-over there).
        denom = small.tile([128, 8], F32, tag="denom", name="denom")
        recip = small.tile([128, 8], F32, tag="recip", name="recip")
        pva = pav[:, 0:260].rearrange("p (a b) -> p a b", b=65)
        pvb = pav[:, 512:772].rearrange("p (a b) -> p a b", b=65)
        nc.vector.tensor_copy(denom[:, 0:4], pva[:, :, 64])
        nc.vector.tensor_copy(denom[:, 4:8], pvb[:, :, 64])
        nc.vector.reciprocal_approx_fast(out=recip[:], in_=denom[:])

        def dst_of(qb):
            return attn_out[h // 2][:, qb * 128 + (h % 2) * 64:
                                    qb * 128 + (h % 2) * 64 + 64]

        pavs = small.tile([128, 520], F32, tag="pavs", name="pavs")
        nc.vector.tensor_copy(pavs[:, 0:260], pav[:, 0:260])
        nc.vector.tensor_copy(pavs[:, 260:520], pav[:, 512:772])
        for qb in range(8):
            eng = nc.vector if qb % 2 == 0 else nc.gpsimd
            eng.tensor_scalar(dst_of(qb), pavs[:, qb * 65:qb * 65 + 64],
                              recip[:, qb:qb + 1], None, MUL)
            if last and qb == 1:
                transp(2, half=0)
            elif last and qb == 3:
                transp(2, half=1)
        if last:
            transp(2, half=2)

    # ---- program ----
    # head 0 prologue: the ACT engine is idle until the first exp, so split
    # the extraction/rel copies across ACT+DVE to reach S(0,0) as early as
    # possible; V projections are injected into ladder 0 instead.
    pq = phase_qk(0, (0, 1, 2))
    phase_qk(0, (3, 4, 5))
    _mark(nc, "extract0")
    # head 0 only: ACT is idle until the first exp, so the k-extract and half
    # the q-extract run there; every read of the qk psum must finish before
    # the rel matmuls can write it (whole-tile WAR tracking).
    nc.scalar.activation(rhs_c[0][0:64, :], pq[64:128, :], IDENT,
                         bias=qkb_sb[64:128, 0:1])
    nc.vector.tensor_scalar(lhsT[0][0:64, 0:512], pq[0:64, 0:512],
                            0.125, qkb_sb[0:64, 0:1], MUL, ADD)
    nc.scalar.activation(lhsT[0][0:64, 512:1024], pq[0:64, 512:1024], IDENT,
                         bias=qkb_sb[0:64, 0:1], scale=0.125)
    phase_v(0, 0)
    phase_rel_h(0, pq)
    phase_rel_w(0, pq)
    _mark(nc, "relc0")
    nc.scalar.copy(lhsT[0][64:96, :], pq[0:32, :])
    phase_rel_copy_w(0, pq)
    phase_v(1, 0)
    phase_v(2, 0)

    state = {"pq": pq}

    def transp(j, half=None):
        # XBAR dma-transpose of a pair-of-heads block: [128, 1024] input,
        # 3D output = per-token-tile [128,128] transposed blocks. Issued as
        # soon as heads 2j, 2j+1 are normalized (in halves for the last pair
        # so the projection can start after the first four norms).
        _mark(nc, f"transp{j}")
        pl4 = proj_lhsT[j][:].rearrange("p (m t) -> p m t", t=128)
        if half is None:
            nc.sync.dma_start_transpose(pl4, attn_out[j][:])
        else:
            q0, q1 = [(0, 2), (2, 4), (4, 8)][half]
            nc.sync.dma_start_transpose(pl4[:, q0:q1, :],
                                        attn_out[j][:, q0 * 128:q1 * 128])

    for h in range(HPC):
        nh = h + 1

        def inject(kb, h=h, nh=nh):
            if h == 0 and kb <= 4:
                phase_v(kb + 3)  # V-proj m=3..7 interleaved into ladder 0
            if nh >= HPC:
                return
            if kb == 0:
                state["pq"] = phase_qk(nh, (0, 1, 2))
            elif kb == 1:
                phase_qk(nh, (3, 4, 5))
            elif kb == 2:
                phase_extract(nh, state["pq"])
            elif kb == 4:
                phase_rel_h(nh, state["pq"])
            elif kb == 5:
                phase_rel_w(nh, state["pq"])
                phase_rel_copy_h(nh, state["pq"])
            elif kb == 6:
                phase_rel_copy_w(nh, state["pq"])

        pav = ladder(h, inject)
        tail(h, pav, last=(h == HPC - 1))
        if h in (1, 3):
            transp(h // 2)

    for m in range(8):
        _mark(nc, f"proj{m}")
        pool, tag = [(ps_s, "ps"), (ps_qk, "pqk"), (ps_s, "ps"), (ps_av, "pav")][m % 4]
        pp = pool.tile([128, DIM], F32, tag=tag, name="pp")
        for t in range(3):
            for n0, nw in ((0, 512), (512, 256)):
                nc.tensor.matmul(pp[:, n0:n0 + nw],
                                 proj_lhsT[t][:, m * 128:(m + 1) * 128],
                                 wp_sb[:, t, n0:n0 + nw],
                                 start=(t == 0), stop=(t == 2))
        osb = outp.tile([128, DIM], BF16, tag="osb", name="osb")
        nc.scalar.copy(osb[:, 0:384], pp[:, 0:384])
        nc.vector.tensor_copy(osb[:, 384:768], pp[:, 384:768])
        eng = nc.sync if m % 2 == 0 else nc.scalar
        eng.dma_start(out_d[m * 128:(m + 1) * 128, :], osb[:])


def _host_prep(x, qkv_w, qkv_b, proj_w, proj_b, rel_pos_h, rel_pos_w):
    bf = ml_dtypes.bfloat16
    idx_h = np.arange(H)[:, None] - np.arange(H)[None, :] + (H - 1)
    idx_w = np.arange(W)[:, None] - np.arange(W)[None, :] + (W - 1)
    Rh = rel_pos_h[idx_h]  # [qh, kh, c]
    Rw = rel_pos_w[idx_w]  # [qw, kw, c]
    rhT8 = np.ascontiguousarray((8.0 * Rh).transpose(2, 0, 1).reshape(HD, H * H)).astype(bf)
    rwT8 = np.ascontiguousarray((8.0 * Rw).transpose(2, 0, 1).reshape(HD, W * W)).astype(bf)
    kt = np.arange(N)
    ec = np.zeros((64, N), np.float32)
    ec[:32] = (np.arange(32)[:, None] == (kt // 32)[None, :])
    ec[32:] = (np.arange(32)[:, None] == (kt % 32)[None, :])
    ec = ec.astype(bf)

    in_maps = []
    for core in range(NCORES):
        b = core // 2
        h0 = (core % 2) * HPC
        xTc = np.ascontiguousarray(x[b].reshape(N, DIM).T).astype(bf)
        wqkc = np.zeros((DIM, HPC * 128), np.float32)
        wvc = np.zeros((DIM, HPC * 64), np.float32)
        wpc = np.zeros((HPC * HD, DIM), np.float32)
        qkbc = np.zeros((128, HPC), np.float32)
        for h in range(HPC):
            g = h0 + h
            wqkc[:, h * 128:h * 128 + 64] = qkv_w[g * HD:(g + 1) * HD].T
            wqkc[:, h * 128 + 64:h * 128 + 128] = qkv_w[DIM + g * HD:DIM + (g + 1) * HD].T
            wvc[:, h * 64:(h + 1) * 64] = qkv_w[2 * DIM + g * HD:2 * DIM + (g + 1) * HD].T
            wpc[h * HD:(h + 1) * HD, :] = proj_w[:, g * HD:(g + 1) * HD].T
            qkbc[0:64, h] = qkv_b[g * HD:(g + 1) * HD] * 0.125
            qkbc[64:128, h] = qkv_b[DIM + g * HD:DIM + (g + 1) * HD]
        in_maps.append({
            "xT": xTc, "wqk": wqkc.astype(bf), "wv": wvc.astype(bf),
            "wp": wpc.astype(bf), "rhT": rhT8, "rwT": rwT8, "ecomb": ec,
            "qkb": qkbc,
        })
    return in_maps


def kernel(x, qkv_w, qkv_b, proj_w, proj_b, rel_pos_h, rel_pos_w, _trace=False):
    x = np.asarray(x, np.float32)
    qkv_w = np.asarray(qkv_w, np.float32)
    qkv_b = np.asarray(qkv_b, np.float32)
    proj_w = np.asarray(proj_w, np.float32)
    proj_b = np.asarray(proj_b, np.float32)
    rel_pos_h = np.asarray(rel_pos_h, np.float32)
    rel_pos_w = np.asarray(rel_pos_w, np.float32)

    in_maps = _host_prep(x, qkv_w, qkv_b, proj_w, proj_b, rel_pos_h, rel_pos_w)
    if "nc" not in _cache:
        _cache["nc"] = build_program()
    nc = _cache["nc"]
    res = run_bass_kernel_spmd(nc, in_maps, core_ids=list(range(NCORES)),
                               trace=_trace)
    parts = [np.asarray(r["out_part"], np.float32) for r in res.results]
    # v-bias enters the output as a constant row: bv @ proj_w.T (attn rows sum
    # to one), folded here together with proj_b.
    bias_row = proj_b + qkv_b[2 * DIM:] @ proj_w.T
    out = np.zeros((B, N, DIM), np.float32)
    for b in range(B):
        out[b] = parts[2 * b] + parts[2 * b + 1] + bias_row
    if _trace:
        kernel.last_results = res
    return out.reshape(B, H, W, DIM)
